# revision 3
# baseline (speedup 1.0000x reference)
"""Bass/Tile TRN2 kernel for per-model-batched causal self-attention.

Problem: x[M,B,S,D], qkv_w[M,D,3D], proj_w[M,D,D] -> out[M,B,S,D]
M=8 models sharded across 8 NeuronCores (embarrassingly parallel).

Per-core design (model m):
  xT      = PE-transpose(x_b)                       [D,S] f32
  qkT     = wqkv[:, :1024].T-proj via fp32r matmul  [1024,S] f32 (q^T,k^T rows)
  V       = x @ wqkv[:, 1024:]  (natural)           [S,512] -> bf16, +ones col
  st[k,q] = K @ Q^T  (fp32r, causal-trimmed)        PSUM f32
  p       = exp(st/8)  (ScalarE, bf16 out), diag blocks masked by tri01 mul
  y_aug   = p.T @ V_aug (bf16)  -> y[q,d] + softmax sums in col 64 (PSUM)
  y       = y_aug * (1/sums)  per-partition scalar  [S,D]
  ynT     = PE-transpose(y)                          [D,S]
  out     = ynT.T @ wproj (fp32r or bf16)
"""

import sys

if "/opt/trn_rl_repo" not in sys.path:
    sys.path.insert(0, "/opt/trn_rl_repo")

import numpy as np

import concourse.bass as bass
import concourse.mybir as mybir
import concourse.tile as tile
from concourse import bacc, bass_utils
from concourse.masks import make_identity, make_upper_triangular

M, B, S, D, H = 8, 4, 512, 512, 8
HD = D // H  # 64
F32 = mybir.dt.float32
F32R = mybir.dt.float32r
BF16 = mybir.dt.bfloat16

# --- knobs ---
PROJ_F32 = True  # final projection in fp32r (True) or bf16 (False)
N_CORES = 8

_cache = {}


def _r(ap):
    return ap.bitcast(F32R)


def build_nc():
    nc = bacc.Bacc("TRN2", target_bir_lowering=False, debug=False)

    x_d = nc.dram_tensor("x", [B, S, D], F32, kind="ExternalInput")
    wqkv_d = nc.dram_tensor("wqkv", [D, 3 * D], F32, kind="ExternalInput")
    wproj_d = nc.dram_tensor("wproj", [D, D], F32, kind="ExternalInput")
    out_d = nc.dram_tensor("out", [B, S, D], F32, kind="ExternalOutput")

    pdt = F32 if PROJ_F32 else BF16
    ytdt = F32R if PROJ_F32 else BF16

    with tile.TileContext(nc) as tc:
        with (
            tc.tile_pool(name="singles", bufs=1) as singles,
            tc.tile_pool(name="xp", bufs=2) as xpool,
            tc.tile_pool(name="xtp", bufs=2) as xtpool,
            tc.tile_pool(name="qk", bufs=2) as qkpool,
            tc.tile_pool(name="vp", bufs=2) as vpool,
            tc.tile_pool(name="se", bufs=2) as sepool,
            tc.tile_pool(name="yp", bufs=2) as ypool,
            tc.tile_pool(name="ytp", bufs=2) as ytpool,
            tc.tile_pool(name="op", bufs=3) as opool,
            tc.tile_pool(name="rp", bufs=2) as rpool,
            tc.tile_pool(name="ps_mm", bufs=2, space=bass.MemorySpace.PSUM) as ps_mm,
            tc.tile_pool(name="ps_st", bufs=2, space=bass.MemorySpace.PSUM) as ps_st,
            tc.tile_pool(name="ps_y", bufs=1, space=bass.MemorySpace.PSUM) as ps_y,
        ):
            # ---- constants & weights (once) ----
            ident = singles.tile([128, 128], F32)
            make_identity(nc, ident[:])
            tri2 = singles.tile([128, 2, 128], BF16)  # upper-tri(incl diag) keep mask, x2
            make_upper_triangular(nc, tri2[:, 0, :], val=1.0, diag=True)
            nc.gpsimd.tensor_copy(out=tri2[:, 1, :], in_=tri2[:, 0, :])

            wqkv = singles.tile([128, 4, 3 * D], F32R)
            nc.sync.dma_start(
                out=wqkv[:],
                in_=wqkv_d.ap().bitcast(F32R).rearrange("(c p) o -> p c o", p=128),
            )
            wproj = singles.tile([128, 4, D], F32R if PROJ_F32 else F32)
            nc.sync.dma_start(
                out=wproj[:],
                in_=wproj_d.ap()
                .bitcast(F32R if PROJ_F32 else F32)
                .rearrange("(c p) o -> p c o", p=128),
            )
            if not PROJ_F32:
                wproj_c = singles.tile([128, 4, D], BF16)
                nc.vector.tensor_copy(out=wproj_c[:], in_=wproj[:])
            else:
                wproj_c = wproj

            for b in range(B):
                # ---- load x_b ----
                x_sb = xpool.tile([128, 4, D], F32, tag="x")  # [p, s_tile, d]
                nc.sync.dma_start(
                    out=x_sb[:], in_=x_d.ap()[b].rearrange("(t p) d -> p t d", p=128)
                )

                # ---- xT[d, s] via PE transpose ----
                xT = []
                for dc in range(4):
                    tp = ps_mm.tile([128, 512], F32, tag="mm")
                    for st in range(4):
                        nc.tensor.transpose(
                            tp[:, st * 128 : (st + 1) * 128],
                            x_sb[:, st, dc * 128 : (dc + 1) * 128],
                            ident[:],
                        )
                    xt = xtpool.tile([128, 512], F32R, tag=f"xt{dc}")
                    nc.any.tensor_copy(out=xt[:], in_=tp[:])
                    xT.append(xt)

                # ---- qkT[o, s] = wqkv[:, :1024].T @ xT  (fp32r) ----
                qkT = []
                for mt in range(8):
                    mp = ps_mm.tile([128, 512], F32, tag="mm")
                    for dc in range(4):
                        nc.tensor.matmul(
                            mp[:],
                            wqkv[:, dc, mt * 128 : (mt + 1) * 128],
                            xT[dc][:],
                            start=(dc == 0),
                            stop=(dc == 3),
                        )
                    qk = qkpool.tile([128, 512], F32R, tag=f"qk{mt}")
                    nc.any.tensor_copy(out=qk[:], in_=mp[:])
                    qkT.append(qk)

                # ---- V[s, o'] natural (fp32r) + ones col -> bf16 V_aug ----
                v_sb = vpool.tile([128, 4, H, 66], BF16, tag="v")  # [p, kt, h, hd+ones+pad]
                nc.gpsimd.memset(v_sb[:, :, :, 64:65], 1.0)
                for st in range(4):
                    vp_ps = ps_mm.tile([128, 512], F32, tag="mm")
                    for dc in range(4):
                        nc.tensor.matmul(
                            vp_ps[:],
                            xT[dc][:, st * 128 : (st + 1) * 128],
                            wqkv[:, dc, 1024:1536],
                            start=(dc == 0),
                            stop=(dc == 3),
                        )
                    nc.any.tensor_copy(
                        out=v_sb[:, st, :, 0:64],
                        in_=vp_ps[:].rearrange("p (h e) -> p h e", h=H),
                    )

                # ---- attention, head-pairs ----
                y_sb = [
                    ypool.tile([128, 512], pdt, tag=f"y{qt}", name=f"ysb{qt}")
                    for qt in range(4)
                ]
                for hg in range(4):
                    h0, h1 = 2 * hg, 2 * hg + 1
                    # scores + exp + mask, per k-chunk
                    se = sepool.tile([128, 4, 2, 512], BF16, tag="se")  # [p, kt, hi, q]
                    for kt in range(4):
                        off = 128 * kt if kt < 3 else 256  # fp32r needs N>=256
                        offe = 128 * kt
                        stp = ps_st.tile([128, 1024], F32, tag="st")
                        for hi, h in enumerate((h0, h1)):
                            mtq, poq = h // 2, 64 * (h % 2)
                            mtk, pok = 4 + h // 2, 64 * (h % 2)
                            nc.tensor.matmul(
                                stp[:, hi * 512 + off : hi * 512 + 512],
                                qkT[mtk][pok : pok + 64, kt * 128 : (kt + 1) * 128],
                                qkT[mtq][poq : poq + 64, off:512],
                                start=True,
                                stop=True,
                            )
                        nc.scalar.activation(
                            out=se[:, kt, :, offe:],
                            in_=stp[:].rearrange("p (hh q) -> p hh q", hh=2)[:, :, offe:],
                            func=mybir.ActivationFunctionType.Exp,
                            scale=1.0 / np.sqrt(HD),
                        )
                        # mask the diagonal block (strict lower triangle -> 0)
                        nc.vector.tensor_mul(
                            out=se[:, kt, :, offe : offe + 128],
                            in0=se[:, kt, :, offe : offe + 128],
                            in1=tri2[:],
                        )

                    # y_aug[q, 65] = sum_kt p[kt].T @ V_aug[kt]
                    yp = ps_y.tile([128, 1024], F32, tag="y")
                    for hi, h in enumerate((h0, h1)):
                        for qt in range(4):
                            base = hi * 512 + qt * 65
                            for kt in range(qt + 1):
                                nc.tensor.matmul(
                                    yp[:, base : base + 65],
                                    se[:, kt, hi, qt * 128 : (qt + 1) * 128],
                                    v_sb[:, kt, h, 0:65],
                                    start=(kt == 0),
                                    stop=(kt == qt),
                                )
                    # softmax denominators -> reciprocals
                    rs = rpool.tile([128, 2, 4], F32, tag="rs")
                    nc.vector.reciprocal(
                        out=rs[:],
                        in_=yp[:].rearrange("p (hh q) -> p hh q", hh=2)[:, :, 64:260:65],
                    )
                    # normalize + scatter into y_sb[qt][:, 64h:64h+64]
                    for hi, h in enumerate((h0, h1)):
                        for qt in range(4):
                            base = hi * 512 + qt * 65
                            nc.any.tensor_scalar_mul(
                                y_sb[qt][:, 64 * h : 64 * h + 64],
                                yp[:, base : base + 64],
                                rs[:, hi, qt : qt + 1],
                            )

                # ---- ynT via PE transpose ----
                ynT = []
                for dc in range(4):
                    tp = ps_mm.tile([128, 512], F32, tag="mm")
                    for qt in range(4):
                        nc.tensor.transpose(
                            tp[:, qt * 128 : (qt + 1) * 128],
                            y_sb[qt][:, dc * 128 : (dc + 1) * 128],
                            ident[:],
                        )
                    yt = ytpool.tile([128, 512], ytdt, tag=f"yt{dc}")
                    nc.any.tensor_copy(out=yt[:], in_=tp[:])
                    ynT.append(yt)

                # ---- out = ynT.T @ wproj ----
                for qt in range(4):
                    op_ps = ps_mm.tile([128, 512], F32, tag="mm")
                    for dc in range(4):
                        lhs = ynT[dc][:, qt * 128 : (qt + 1) * 128]
                        rhs = wproj_c[:, dc, :]
                        nc.tensor.matmul(
                            op_ps[:], lhs, rhs, start=(dc == 0), stop=(dc == 3)
                        )
                    ob = opool.tile([128, 512], F32, tag="ob")
                    nc.any.tensor_copy(out=ob[:], in_=op_ps[:])
                    nc.sync.dma_start(
                        out=out_d.ap()[b, qt * 128 : (qt + 1) * 128, :], in_=ob[:]
                    )

    nc.compile()
    return nc


def kernel(x, qkv_weight, proj_weight):
    if "nc" not in _cache:
        _cache["nc"] = build_nc()
    nc = _cache["nc"]
    in_maps = [
        {
            "x": np.ascontiguousarray(x[m], dtype=np.float32),
            "wqkv": np.ascontiguousarray(qkv_weight[m], dtype=np.float32),
            "wproj": np.ascontiguousarray(proj_weight[m], dtype=np.float32),
        }
        for m in range(M)
    ]
    res = bass_utils.run_bass_kernel_spmd(nc, in_maps, core_ids=list(range(N_CORES)))
    return np.stack([res.results[m]["out"] for m in range(M)]).astype(np.float32)


# revision 17
# speedup vs baseline: 1.1244x; 1.1244x over previous
"""Bass/Tile TRN2 kernel for per-model-batched causal self-attention.

Problem: x[M,B,S,D], qkv_w[M,D,3D], proj_w[M,D,D] -> out[M,B,S,D]
M=8 models sharded across 8 NeuronCores (embarrassingly parallel).

Per-core design (model m):
  xT      = PE-transpose(x_b)                       [D,S] f32
  qkT     = wqkv[:, :1024].T-proj via fp32r matmul  [1024,S] f32 (q^T,k^T rows)
  V       = x @ wqkv[:, 1024:]  (natural)           [S,512] -> bf16, +ones col
  st[k,q] = K @ Q^T  (fp32r, causal-trimmed)        PSUM f32
  p       = exp(st/8)  (ScalarE, bf16 out), diag blocks masked by tri01 mul
  y_aug   = p.T @ V_aug (bf16)  -> y[q,d] + softmax sums in col 64 (PSUM)
  y       = y_aug * (1/sums)  per-partition scalar  [S,D]
  ynT     = PE-transpose(y)                          [D,S]
  out     = ynT.T @ wproj (fp32r or bf16)
"""

import sys

if "/opt/trn_rl_repo" not in sys.path:
    sys.path.insert(0, "/opt/trn_rl_repo")

import numpy as np

import concourse.bass as bass
import concourse.mybir as mybir
import concourse.tile as tile
from concourse import bacc, bass_utils
from concourse.masks import make_identity, make_upper_triangular

M, B, S, D, H = 8, 4, 512, 512, 8
HD = D // H  # 64
F32 = mybir.dt.float32
F32R = mybir.dt.float32r
BF16 = mybir.dt.bfloat16

# --- knobs ---
PROJ_F32 = True  # final projection in fp32r (True) or bf16 (False)
N_CORES = 8

_cache = {}


def _r(ap):
    return ap.bitcast(F32R)


def build_nc():
    nc = bacc.Bacc("TRN2", target_bir_lowering=False, debug=False)

    x_d = nc.dram_tensor("x", [B, S, D], F32, kind="ExternalInput")
    wqkv_d = nc.dram_tensor("wqkv", [D, 3 * D], F32, kind="ExternalInput")
    wproj_d = nc.dram_tensor("wproj", [D, D], F32, kind="ExternalInput")
    out_d = nc.dram_tensor("out", [B, S, D], F32, kind="ExternalOutput")

    pdt = F32R if PROJ_F32 else BF16
    ytdt = F32R if PROJ_F32 else BF16

    with tile.TileContext(nc) as tc:
        with (
            tc.tile_pool(name="singles", bufs=1) as singles,
            tc.tile_pool(name="xp", bufs=2) as xpool,
            tc.tile_pool(name="xtp", bufs=2) as xtpool,
            tc.tile_pool(name="qk", bufs=2) as qkpool,
            tc.tile_pool(name="vp", bufs=2) as vpool,
            tc.tile_pool(name="se", bufs=2) as sepool,
            tc.tile_pool(name="yp", bufs=2) as ypool,
            tc.tile_pool(name="ytp", bufs=2) as ytpool,
            tc.tile_pool(name="op", bufs=3) as opool,
            tc.tile_pool(name="rp", bufs=2) as rpool,
            tc.tile_pool(name="ps_mm", bufs=2, space=bass.MemorySpace.PSUM) as ps_mm,
            tc.tile_pool(name="ps_att", bufs=3, space=bass.MemorySpace.PSUM) as ps_att,
        ):
            # ---- constants & weights (once) ----
            ident = singles.tile([128, 128], F32)
            make_identity(nc, ident[:])
            ident_r = singles.tile([128, 128], F32R)
            nc.vector.tensor_copy(out=ident_r[:], in_=ident[:])
            ident_y = ident_r
            if not PROJ_F32:
                ident_y = singles.tile([128, 128], BF16, name="identb")
                nc.vector.tensor_copy(out=ident_y[:], in_=ident[:])
            tri2 = singles.tile([128, 2, 128], BF16)  # upper-tri(incl diag) keep mask, x2
            make_upper_triangular(nc, tri2[:, 0, :], val=1.0, diag=True)
            nc.gpsimd.tensor_copy(out=tri2[:, 1, :], in_=tri2[:, 0, :])

            wqkv = singles.tile([128, 4, 3 * D], F32R)
            wproj = singles.tile([128, 4, D], F32R if PROJ_F32 else F32)
            wproj_c = wproj

            for b in range(B):
                # ---- load x_b ----
                x_sb = xpool.tile([128, 4, D], F32R, tag="x")  # [p, s_tile, d]
                nc.sync.dma_start(
                    out=x_sb[:], in_=x_d.ap().bitcast(F32R)[b].rearrange("(t p) d -> p t d", p=128)
                )
                if b == 0:
                    for dc in range(4):
                        nc.sync.dma_start(
                            out=wqkv[:, dc, :],
                            in_=wqkv_d.ap().bitcast(F32R)[dc * 128 : (dc + 1) * 128, :],
                        )
                    nc.sync.dma_start(
                        out=wproj[:],
                        in_=wproj_d.ap()
                        .bitcast(F32R if PROJ_F32 else F32)
                        .rearrange("(c p) o -> p c o", p=128),
                    )
                    if not PROJ_F32:
                        wproj_cc = singles.tile([128, 4, D], BF16, name="wprojc")
                        nc.vector.tensor_copy(out=wproj_cc[:], in_=wproj[:])
                        wproj_c = wproj_cc

                # ---- xT[d, s] via PE transpose ----
                xT = []
                for dc in range(4):
                    tp = ps_mm.tile([128, 512], F32, tag="mm")
                    for st in range(4):
                        nc.tensor.transpose(
                            tp[:, st * 128 : (st + 1) * 128].bitcast(F32R),
                            x_sb[:, st, dc * 128 : (dc + 1) * 128],
                            ident_r[:],
                        )
                    xt = xtpool.tile([128, 512], F32R, tag=f"xt{dc}")
                    nc.vector.tensor_copy(out=xt[:], in_=tp[:])
                    xT.append(xt)

                # ---- qkT[o, s] = wqkv[:, :1024].T @ xT  (fp32r) ----
                qkT = []
                for mt in range(8):
                    mp = ps_mm.tile([128, 512], F32, tag="mm")
                    for dc in range(4):
                        nc.tensor.matmul(
                            mp[:],
                            wqkv[:, dc, mt * 128 : (mt + 1) * 128],
                            xT[dc][:],
                            start=(dc == 0),
                            stop=(dc == 3),
                        )
                    qk = qkpool.tile([128, 512], F32R, tag=f"qk{mt}")
                    nc.vector.tensor_copy(out=qk[:], in_=mp[:])
                    qkT.append(qk)

                # ---- V[s, o'] natural (fp32r) + ones col -> bf16 V_aug ----
                v_sb = vpool.tile([128, 4, H, 66], BF16, tag="v")  # [p, kt, h, hd+ones+pad]
                nc.gpsimd.memset(v_sb[:, :, :, 64:65], 1.0)
                for st in range(4):
                    vp_ps = ps_mm.tile([128, 512], F32, tag="mm")
                    for dc in range(4):
                        nc.tensor.matmul(
                            vp_ps[:],
                            xT[dc][:, st * 128 : (st + 1) * 128],
                            wqkv[:, dc, 1024:1536],
                            start=(dc == 0),
                            stop=(dc == 3),
                        )
                    nc.scalar.copy(
                        out=v_sb[:, st, :, 0:64],
                        in_=vp_ps[:].rearrange("p (h e) -> p h e", h=H),
                    )

                # ---- attention, head-pairs, software-pipelined ----
                y_sb = [
                    ypool.tile([128, 512], pdt, tag=f"y{qt}", name=f"ysb{qt}")
                    for qt in range(4)
                ]
                ynT = []

                def emit_scores(hg):
                    h0, h1 = 2 * hg, 2 * hg + 1
                    se = sepool.tile(
                        [128, 4, 2, 512], BF16, tag="se", name="se"
                    )  # [p, kt, hi, q]
                    for kt in range(4):
                        off = 128 * kt if kt < 3 else 256  # fp32r needs N>=256
                        offe = 128 * kt
                        stp = ps_att.tile([128, 1024], F32, tag="att", name="stp")
                        for hi, h in enumerate((h0, h1)):
                            mtq, poq = h // 2, 64 * (h % 2)
                            mtk, pok = 4 + h // 2, 64 * (h % 2)
                            nc.tensor.matmul(
                                stp[:, hi * 512 + off : hi * 512 + 512],
                                qkT[mtk][pok : pok + 64, kt * 128 : (kt + 1) * 128],
                                qkT[mtq][poq : poq + 64, off:512],
                                start=True,
                                stop=True,
                            )
                        nc.scalar.activation(
                            out=se[:, kt, :, offe:],
                            in_=stp[:].rearrange("p (hh q) -> p hh q", hh=2)[
                                :, :, offe:
                            ],
                            func=mybir.ActivationFunctionType.Exp,
                            scale=1.0 / np.sqrt(HD),
                        )
                        # mask the diagonal block (strict lower triangle -> 0)
                        nc.vector.tensor_mul(
                            out=se[:, kt, :, offe : offe + 128],
                            in0=se[:, kt, :, offe : offe + 128],
                            in1=tri2[:],
                        )
                    return se

                def emit_y(hg, se):
                    h0, h1 = 2 * hg, 2 * hg + 1
                    # y_aug[q, 65] = sum_kt p[kt].T @ V_aug[kt]
                    yp = ps_att.tile([128, 1024], F32, tag="att", name="yp")
                    for hi, h in enumerate((h0, h1)):
                        for qt in range(4):
                            base = hi * 512 + qt * 65
                            for kt in range(qt + 1):
                                nc.tensor.matmul(
                                    yp[:, base : base + 65],
                                    se[:, kt, hi, qt * 128 : (qt + 1) * 128],
                                    v_sb[:, kt, h, 0:65],
                                    start=(kt == 0),
                                    stop=(kt == qt),
                                )
                    # softmax denominators -> reciprocals
                    rs = rpool.tile([128, 2, 4], F32, tag="rs", name="rs")
                    nc.vector.reciprocal_approx_fast(
                        out=rs[:],
                        in_=yp[:].rearrange("p (hh q) -> p hh q", hh=2)[
                            :, :, 64:260:65
                        ],
                    )
                    # normalize + scatter into y_sb[qt][:, 64h:64h+64]
                    for hi, h in enumerate((h0, h1)):
                        for qt in range(4):
                            base = hi * 512 + qt * 65
                            nc.vector.tensor_scalar_mul(
                                y_sb[qt][:, 64 * h : 64 * h + 64],
                                yp[:, base : base + 64],
                                rs[:, hi, qt : qt + 1],
                            )
                    # yT transpose for the d-slice this head-pair completed
                    dc = hg
                    tp = ps_mm.tile([128, 512], F32, tag="mm", name="tpy")
                    for qt in range(4):
                        nc.tensor.transpose(
                            tp[:, qt * 128 : (qt + 1) * 128].bitcast(pdt),
                            y_sb[qt][:, dc * 128 : (dc + 1) * 128],
                            ident_y[:],
                        )
                    yt = ytpool.tile([128, 512], ytdt, tag=f"yt{dc}", name=f"yt{dc}")
                    nc.scalar.copy(out=yt[:], in_=tp[:])
                    ynT.append(yt)

                se_prev = emit_scores(0)
                for hg in range(4):
                    se_next = emit_scores(hg + 1) if hg + 1 < 4 else None
                    emit_y(hg, se_prev)
                    se_prev = se_next

                # ---- out = ynT.T @ wproj ----
                for qt in range(4):
                    op_ps = ps_mm.tile([128, 512], F32, tag="mm")
                    for dc in range(4):
                        lhs = ynT[dc][:, qt * 128 : (qt + 1) * 128]
                        rhs = wproj_c[:, dc, :]
                        nc.tensor.matmul(
                            op_ps[:], lhs, rhs, start=(dc == 0), stop=(dc == 3)
                        )
                    ob = opool.tile([128, 512], F32, tag="ob")
                    nc.vector.tensor_copy(out=ob[:], in_=op_ps[:])
                    nc.sync.dma_start(
                        out=out_d.ap()[b, qt * 128 : (qt + 1) * 128, :], in_=ob[:]
                    )

    nc.compile()
    return nc


def kernel(x, qkv_weight, proj_weight):
    if "nc" not in _cache:
        _cache["nc"] = build_nc()
    nc = _cache["nc"]
    in_maps = [
        {
            "x": np.ascontiguousarray(x[m], dtype=np.float32),
            "wqkv": np.ascontiguousarray(qkv_weight[m], dtype=np.float32),
            "wproj": np.ascontiguousarray(proj_weight[m], dtype=np.float32),
        }
        for m in range(M)
    ]
    res = bass_utils.run_bass_kernel_spmd(nc, in_maps, core_ids=list(range(N_CORES)))
    return np.stack([res.results[m]["out"] for m in range(M)]).astype(np.float32)
